# revision 1
# baseline (speedup 1.0000x reference)
"""Causal self-attention kernel for 8 Trainium2 NeuronCores.

Problem: B=4, S=2048, D=1024, H=16, HD=64 (fp32).
  qkv = x @ w_qkv.T ; per-head causal softmax attention ; out @ w_proj.T

Sharding: core c handles batch b = c//2 and head-half hh = c%2 (8 heads).
Each core computes its 8 heads' attention and a partial output projection
(w_proj column slice); the host sums the two partials per batch.

v1 pipeline (vs baseline):
  - all weight DMAs prefetched (2-buf group slices); no mid-kernel PE
    stalls on DMA -> HAM clock gate stays warm (2.4 GHz)
  - Q/K proj with weights stationary (dk-outer, s-windows moving)
  - group g+1 Q/K proj matmuls emission-interleaved into attention(g)'s
    qb loop so the in-order PE fills exp(ACT)-paced gaps
  - epilogue uses reciprocal_approx_fast (DVE custom op, ~5x)
  - V/P/attention-band in bf16 (AV matmuls bf16; PE rate unchanged)
  - out proj w-stationary in bf16 -> yT [e, s]; host transposes
  - PSUM: psA bufs=2 (4 banks) + proj pool (2) + psO (2) = 8
"""

import sys

if "/opt/trn_rl_repo" not in sys.path:
    sys.path.insert(0, "/opt/trn_rl_repo")

import numpy as np

import concourse.tile as tile
from concourse import bacc, mybir

F32 = mybir.dt.float32
F32R = mybir.dt.float32r
BF16 = mybir.dt.bfloat16
EXP = mybir.ActivationFunctionType.Exp

B, S, D = 4, 2048, 1024
H, HD = 16, 64
P = 128
DT = D // P            # 8 d-tiles (contraction tiles for projections)
NHC = 8                # heads per core
NG = NHC // 2          # head pair-groups per core
QB = 4                 # q-blocks of 512
QW = 512               # q-block width
KT = S // P            # 16 k-tiles
XCH = 8                # xT DMA split chunks (along seq)
SCALE = 1.0 / np.sqrt(HD)

_NC = None


def _build(loop_reps=1):
    nc = bacc.Bacc("TRN2", target_bir_lowering=False, debug=False)

    xT = nc.dram_tensor("xT", [D, S], BF16, kind="ExternalInput")
    wqT = nc.dram_tensor("wqT", [D, 512], BF16, kind="ExternalInput")
    wkT = nc.dram_tensor("wkT", [D, 512], BF16, kind="ExternalInput")
    wvT = nc.dram_tensor("wvT", [D, 512], BF16, kind="ExternalInput")
    wpT = nc.dram_tensor("wpT", [512, D], BF16, kind="ExternalInput")
    yT = nc.dram_tensor("yT", [D, S], BF16, kind="ExternalOutput")

    with tile.TileContext(nc) as tc:
        if loop_reps > 1:
            with tc.For_i(0, loop_reps, 1):
                _body(nc, tc, xT, wqT, wkT, wvT, wpT, yT)
        else:
            _body(nc, tc, xT, wqT, wkT, wvT, wpT, yT)
    nc.compile()
    return nc


def _body(nc, tc, xT, wqT, wkT, wvT, wpT, yT):
    with (
        tc.tile_pool(name="big", bufs=1) as big,
        tc.tile_pool(name="wsl", bufs=2) as wsl,
        tc.tile_pool(name="qk", bufs=2) as qkp,
        tc.tile_pool(name="pfull", bufs=3) as pfp,
        tc.tile_pool(name="pband", bufs=1) as pbp,
        tc.tile_pool(name="small", bufs=2) as sp,
        tc.tile_pool(name="small1", bufs=1) as sp1,
        tc.tile_pool(name="ost", bufs=6) as ostp,
        tc.tile_pool(name="psA", bufs=2, space="PSUM") as psA,
        tc.tile_pool(name="psP", bufs=2, space="PSUM") as psP,
        tc.tile_pool(name="psO", bufs=2, space="PSUM") as psO,
    ):
        # ---- persistent loads -------------------------------------------
        # xT split into seq-chunks so compute can start before the full load;
        # wv + first chunks lead so the V projection starts ~8us in
        xT_sb = big.tile([P, DT, S], BF16, tag="xT")
        xT_src = xT.ap().rearrange("(o p) s -> p o s", p=P)
        xw = S // XCH

        def load_xchunk(c):
            nc.sync.dma_start(
                xT_sb[:, :, c * xw:(c + 1) * xw], xT_src[:, :, c * xw:(c + 1) * xw])

        wvT_sb = big.tile([P, DT, 512], BF16, tag="wv")
        nc.sync.dma_start(wvT_sb, wvT.ap().rearrange("(o p) e -> p o e", p=P))
        load_xchunk(0)
        load_xchunk(1)

        # per-group Q/K weight slices, double-buffered + prefetched
        wq_sb = [None] * NG
        wk_sb = [None] * NG

        def load_wqk(g):
            wq_sb[g] = wsl.tile([P, DT, P], BF16, tag="wq", name=f"wq_{g}")
            nc.sync.dma_start(
                wq_sb[g],
                wqT.ap().rearrange("(o p) e -> p o e", p=P)[:, :, g * P:(g + 1) * P],
            )
            wk_sb[g] = wsl.tile([P, DT, P], BF16, tag="wk", name=f"wk_{g}")
            nc.sync.dma_start(
                wk_sb[g],
                wkT.ap().rearrange("(o p) e -> p o e", p=P)[:, :, g * P:(g + 1) * P],
            )

        load_wqk(0)
        for c in range(2, XCH):
            load_xchunk(c)
        load_wqk(1)

        wpT_sb = big.tile([P, 4, D], BF16, tag="wpT")

        # V with a ones column per head: [P, kt, 8 heads * 65]
        vaug = big.tile([P, KT, NHC * 65], F32R, tag="vaug")
        ones_cols = vaug.rearrange("p t (h c) -> p t h c", c=65)[:, :, :, 64]
        nc.gpsimd.memset(ones_cols.bitcast(F32), 1.0)

        # warm the ACT exp table while DMAs are in flight (first call to a
        # new activation set costs ~2.7us; keep it off the attention path)
        zwarm = sp1.tile([1, 1], F32, tag="zwarm")
        nc.scalar.activation(zwarm, wvT_sb[0:1, 0, 0:1], EXP, scale=SCALE)

        # ---- V projection (all 8 heads, one seq-chunk per psum tile) -----
        def emit_vproj_chunk(sp2):
            pv = psA.tile([P, 2, QW], F32, tag="mm", name=f"pv_{sp2}")
            for half in range(2):
                st = 2 * sp2 + half
                for dk in range(DT):
                    nc.tensor.matmul(
                        pv[:, half, :],
                        lhsT=xT_sb[:, dk, st * P:(st + 1) * P],
                        rhs=wvT_sb[:, dk, :],
                        start=(dk == 0), stop=(dk == DT - 1),
                    )
            nc.vector.tensor_copy(
                out=vaug[:, 2 * sp2:2 * sp2 + 2, :]
                    .rearrange("p t (h c) -> p t h c", c=65)[:, :, :, 0:64],
                in_=pv.rearrange("p t (h c) -> p t h c", c=64),
            )

        for sp2 in range(4):
            emit_vproj_chunk(sp2)

        # wp DMA late (after the x/wv/wq/wk burst; needed only at the end)
        nc.sync.dma_start(wpT_sb, wpT.ap().rearrange("(t p) e -> p t e", p=P))

        # output accumulator O'[do, q] (do = local_head*64 + hd), normalized
        oall = big.tile([P, NG, S], BF16, tag="oall")

        qT = [None] * NG
        kT = [None] * NG

        # ---- PE filler queue: single-matmul closures (next group's Q/K
        # proj, g3's output proj) drained into ACT-paced attention bubbles;
        # the PE issues in-order, so later-emitted work cannot overtake a
        # stalled AV unless we zipper it in at emission time.
        filler = []

        def fill(n):
            for _ in range(min(n, len(filler))):
                filler.pop(0)[1]()

        def fill_all():
            while filler:
                filler.pop(0)[1]()

        def drain_until(label):
            while filler:
                lb, fn = filler.pop(0)
                fn()
                if lb == label:
                    return

        def push_proj_sub(g, which, half):
            """Queue one 1024-seq half of group g's Q or K projection."""
            w_sb = wq_sb[g] if which == "q" else wk_sb[g]
            if half == 0:
                dst = qkp.tile([P, S], BF16, tag=which, name=f"{which}T_{g}")
                if which == "q":
                    qT[g] = dst
                else:
                    kT[g] = dst
            dst = qT[g] if which == "q" else kT[g]
            for j in range(2):
                sw = 2 * half + j
                box = {}

                def mm(dk, sw=sw, box=box):
                    if dk == 0:
                        box["pt"] = psP.tile(
                            [P, QW], F32, tag="pj",
                            name=f"pj_{which}_{g}_{sw}")
                    nc.tensor.matmul(
                        box["pt"],
                        lhsT=w_sb[:, dk, :],
                        rhs=xT_sb[:, dk, sw * QW:(sw + 1) * QW],
                        start=(dk == 0), stop=(dk == DT - 1),
                    )
                    if dk == DT - 1:
                        nc.vector.tensor_copy(
                            out=dst[:, sw * QW:(sw + 1) * QW], in_=box["pt"])

                for dk in range(DT):
                    filler.append((None, lambda dk=dk, mm=mm: mm(dk)))

        def push_outproj_sw(sw):
            # yT[e, sw-window] = sum_t wpT[:, t, e].T @ oall[:, t, sw-window]
            for eb in range(D // P):
                box = {}

                def mm(t, eb=eb, box=box):
                    if t == 0:
                        box["pt"] = psP.tile(
                            [P, QW], F32, tag="pj", name=f"pfin_{sw}_{eb}")
                    nc.tensor.matmul(
                        box["pt"],
                        lhsT=wpT_sb[:, t, eb * P:(eb + 1) * P],
                        rhs=oall[:, t, sw * QW:(sw + 1) * QW],
                        start=(t == 0), stop=(t == NG - 1),
                    )
                    if t == NG - 1:
                        ot = ostp.tile([P, QW], BF16, tag="ot",
                                       name=f"ot_{sw}_{eb}")
                        nc.vector.tensor_copy(out=ot, in_=box["pt"])
                        nc.sync.dma_start(
                            yT.ap()[eb * P:(eb + 1) * P,
                                    sw * QW:(sw + 1) * QW],
                            ot,
                        )

                for t in range(NG):
                    filler.append((None, lambda t=t, mm=mm: mm(t)))

        # group 0 Q/K projection emitted straight; V chunks 4-7 become
        # filler for attention(g0) (qb0/qb1 only touch k-tiles 0-7 = chunks
        # 0-3; drain_until guards qb2/qb3)
        for which in ("q", "k"):
            for half in range(2):
                push_proj_sub(0, which, half)
        fill_all()
        # emit V chunks 4-7 straight: as filler inside g0's band block their
        # psA allocs would rotate into the band-scores slots and stall on
        # the same exp chain the first AV waits for
        for sp2 in range(4, KT // 2):
            emit_vproj_chunk(sp2)

        # ---- per head-pair-group: attention + zippered filler -----------
        for g in range(NG):
            if g + 2 <= NG - 1:
                load_wqk(g + 2)
            if g + 1 <= NG - 1:
                for which in ("q", "k"):
                    for half in range(2):
                        push_proj_sub(g + 1, which, half)
            qTg, kTg = qT[g], kT[g]

            qb_order = range(QB) if g + 1 <= NG - 1 else range(QB - 1, -1, -1)
            for qb in qb_order:
                nfull = 4 * qb  # full k-tiles 0 .. 4qb-1, then 4 band tiles
                po = [
                    psO.tile([65, QW], F32, tag="po", name=f"po_{g}_{qb}_{hl}")
                    for hl in range(2)
                ]
                pband = pbp.tile([P, 2, 4, QW], F32R, tag="pband",
                                 name=f"pband_{g}_{qb}")

                def scores_full(kt, qb=qb, qTg=qTg, kTg=kTg, g=g):
                    ps2 = psA.tile([P, 2, QW], F32, tag="mm",
                                   name=f"ps_{g}_{qb}_{kt}")
                    for hl in range(2):
                        hp = hl * 64
                        nc.tensor.matmul(
                            ps2[:, hl, :],
                            lhsT=kTg[hp:hp + 64, kt * P:(kt + 1) * P],
                            rhs=qTg[hp:hp + 64, qb * QW:(qb + 1) * QW],
                            start=True, stop=True,
                        )
                    pp = pfp.tile([P, 2, QW], F32R, tag="pf",
                                  name=f"pf_{g}_{qb}_{kt}")
                    nc.scalar.activation(pp, ps2, EXP, scale=SCALE)
                    return pp

                # 1) band scores + exp + per-rel mask first (their ACT/gpsimd
                #    chain overlaps filler + leading full scores)
                for rel in range(4):
                    kt = 4 * qb + rel
                    v0 = P * rel
                    ps2 = psA.tile([P, 2, QW], F32, tag="mm",
                                   name=f"ps_{g}_{qb}_{kt}")
                    for hl in range(2):
                        hp = hl * 64
                        nc.tensor.matmul(
                            ps2[:, hl, v0:],
                            lhsT=kTg[hp:hp + 64, kt * P:(kt + 1) * P],
                            rhs=qTg[hp:hp + 64, qb * QW + v0:(qb + 1) * QW],
                            start=True, stop=True,
                        )
                    nc.scalar.activation(
                        pband[:, :, rel, v0:], ps2[:, :, v0:], EXP,
                        scale=SCALE)
                    w0 = min(v0, QW - 2 * P)  # AV reads from here
                    w1 = QW if rel == 3 else min(v0 + P, QW)
                    nc.gpsimd.affine_select(
                        out=pband[:, :, rel, w0:w1],
                        in_=pband[:, :, rel, w0:w1],
                        compare_op=mybir.AluOpType.is_ge, fill=0.0,
                        base=w0 - P * rel, channel_multiplier=-1,
                        pattern=[[0, 2], [1, w1 - w0]],
                    )
                    fill(1)

                # 2) leading full scores + filler bridge the band-exp chain
                lead = min(2, nfull)
                pps = {kt: scores_full(kt) for kt in range(lead)}
                fill(4)

                # 3) band AVs (rel0 covers all columns => carries start)
                for rel in range(4):
                    kt = 4 * qb + rel
                    av0 = min(P * rel, QW - 2 * P)
                    for hl in range(2):
                        h = 2 * g + hl
                        nc.tensor.matmul(
                            po[hl][:, av0:],
                            lhsT=vaug[:, kt, h * 65:(h + 1) * 65],
                            rhs=pband[:, hl, rel, av0:],
                            start=(rel == 0), stop=(rel == 3 and nfull == 0),
                        )
                    fill(1)

                # 4) remaining full tiles, scores->exp->AV with 1-filler gaps
                for kt in range(nfull):
                    if kt not in pps:
                        pps[kt] = scores_full(kt)
                    if kt + lead < nfull:
                        pps[kt + lead] = scores_full(kt + lead)
                    fill(1)
                    pp = pps.pop(kt)
                    for hl in range(2):
                        h = 2 * g + hl
                        nc.tensor.matmul(
                            po[hl],
                            lhsT=vaug[:, kt, h * 65:(h + 1) * 65],
                            rhs=pp[:, hl, :],
                            start=False, stop=(kt == nfull - 1),
                        )

                for hl in range(2):
                    zrow = sp1.tile([1, QW], F32, tag="zrow",
                                    name=f"zr_{g}_{qb}_{hl}")
                    nc.vector.tensor_copy(out=zrow, in_=po[hl][64:65, :])
                    recip = sp1.tile([1, QW], F32, tag="recip",
                                     name=f"rc_{g}_{qb}_{hl}")
                    nc.vector.reciprocal_approx_fast(recip, zrow)
                    bc = sp.tile([64, QW], F32, tag="bc",
                                 name=f"bc_{g}_{qb}_{hl}")
                    nc.gpsimd.partition_broadcast(bc, recip)
                    nc.vector.tensor_mul(
                        out=oall[hl * 64:(hl + 1) * 64, g, qb * QW:(qb + 1) * QW],
                        in0=po[hl][0:64, :],
                        in1=bc,
                    )

                if g + 1 > NG - 1:
                    # g3: this qb's s-window of the output projection becomes
                    # filler for the remaining (reversed-order) qbs
                    push_outproj_sw(qb)

            fill_all()

def _get_nc():
    global _NC
    if _NC is None:
        _NC = _build()
    return _NC


def _in_maps(x, w_qkv, w_proj):
    from ml_dtypes import bfloat16

    x = np.asarray(x, dtype=np.float32)
    w_qkv = np.asarray(w_qkv, dtype=np.float32)
    w_proj = np.asarray(w_proj, dtype=np.float32)

    maps = []
    for c in range(8):
        b, hh = c // 2, c % 2
        lo, hi = hh * 512, (hh + 1) * 512
        maps.append({
            "xT": np.ascontiguousarray(x[b].T).astype(bfloat16),
            "wqT": np.ascontiguousarray(w_qkv[lo:hi].T).astype(bfloat16),
            "wkT": np.ascontiguousarray(w_qkv[D + lo:D + hi].T).astype(bfloat16),
            "wvT": np.ascontiguousarray(w_qkv[2 * D + lo:2 * D + hi].T).astype(bfloat16),
            "wpT": np.ascontiguousarray(w_proj[:, lo:hi].T).astype(bfloat16),
        })
    return maps


def kernel(x, w_qkv, w_proj):
    from concourse.bass_utils import run_bass_kernel_spmd

    in_maps = _in_maps(x, w_qkv, w_proj)
    res = run_bass_kernel_spmd(_get_nc(), in_maps, core_ids=list(range(8)))
    out = np.empty((B, S, D), dtype=np.float32)
    for b in range(B):
        out[b] = (res.results[2 * b]["yT"].astype(np.float32)
                  + res.results[2 * b + 1]["yT"].astype(np.float32)).T
    return out



# revision 45
# speedup vs baseline: 1.0873x; 1.0873x over previous
"""Causal self-attention kernel for 8 Trainium2 NeuronCores.

Problem: B=4, S=2048, D=1024, H=16, HD=64 (fp32).
  qkv = x @ w_qkv.T ; per-head causal softmax attention ; out @ w_proj.T

Sharding: core c handles batch b = c//2 and head-half hh = c%2 (8 heads).
Each core computes its 8 heads' attention and a partial output projection
(w_proj column slice); the host sums the two partials per batch.

v1 pipeline (vs baseline):
  - all weight DMAs prefetched (2-buf group slices); no mid-kernel PE
    stalls on DMA -> HAM clock gate stays warm (2.4 GHz)
  - Q/K proj with weights stationary (dk-outer, s-windows moving)
  - group g+1 Q/K proj matmuls emission-interleaved into attention(g)'s
    qb loop so the in-order PE fills exp(ACT)-paced gaps
  - epilogue uses reciprocal_approx_fast (DVE custom op, ~5x)
  - V/P/attention-band in bf16 (AV matmuls bf16; PE rate unchanged)
  - out proj w-stationary in bf16 -> yT [e, s]; host transposes
  - PSUM: psA bufs=2 (4 banks) + proj pool (2) + psO (2) = 8

v2 (HW 294us single-shot -> ~269us loop steady-state):
  - copy-first epilogue: po (psum) -> ocp (SBUF) immediately after the AV
    stop so the psO buf frees after one DVE copy; recip/bcast/mul then run
    from SBUF off the next qb's AV critical path (the psO pair-per-qb
    rotation was costing 400-800ns per qb)
  - g3 processes qbs FORWARD so the big qb3 (most fill slots) overlaps the
    bulk of the outproj; the tail is only sw3's outproj
  - wv (1MB) DMA moved to the Activation HWDGE queue, x chunks on SP
  - scores pairs run row-tiled concurrent on HW (auto tile_position from
    base partitions 0/64; measured 336ns/pair vs 947 serial-forced)
  - measured dead ends (microbenched): fp8 DoubleRow scores are
    LDWEIGHTS-bound (559ns/pair, worse than bf16 336); col-tiled M=64 AV
    pairs are 1.64x faster (337 vs 553) but every scheme to recover the
    softmax denominator (z) off the ones-row costs back the gain (quad
    M=1 ones-matmuls: +212ns/pair; DVE partition-tree: +50-85us DVE);
    outproj phase-split into per-t passes trades psum accumulation for
    +74us of DVE copies
"""

import sys

if "/opt/trn_rl_repo" not in sys.path:
    sys.path.insert(0, "/opt/trn_rl_repo")

import numpy as np

import concourse.tile as tile
from concourse import bacc, mybir

F32 = mybir.dt.float32
F32R = mybir.dt.float32r
BF16 = mybir.dt.bfloat16
EXP = mybir.ActivationFunctionType.Exp

B, S, D = 4, 2048, 1024
H, HD = 16, 64
P = 128
DT = D // P            # 8 d-tiles (contraction tiles for projections)
NHC = 8                # heads per core
NG = NHC // 2          # head pair-groups per core
QB = 4                 # q-blocks of 512
QW = 512               # q-block width
KT = S // P            # 16 k-tiles
XCH = 8                # xT DMA split chunks (along seq)
SCALE = 1.0 / np.sqrt(HD)

_NC = None

# experiment flags (set via _build(opts=...)); default {} = baseline
OPTS = {}


def _build(loop_reps=1, opts=None):
    global OPTS
    OPTS = opts or {}
    nc = bacc.Bacc("TRN2", target_bir_lowering=False, debug=False)

    xT = nc.dram_tensor("xT", [D, S], BF16, kind="ExternalInput")
    wqT = nc.dram_tensor("wqT", [D, 512], BF16, kind="ExternalInput")
    wkT = nc.dram_tensor("wkT", [D, 512], BF16, kind="ExternalInput")
    wvT = nc.dram_tensor("wvT", [D, 512], BF16, kind="ExternalInput")
    wpT = nc.dram_tensor("wpT", [512, D], BF16, kind="ExternalInput")
    yT = nc.dram_tensor("yT", [D, S], BF16, kind="ExternalOutput")

    with tile.TileContext(nc) as tc:
        if loop_reps > 1:
            with tc.For_i(0, loop_reps, 1):
                _body(nc, tc, xT, wqT, wkT, wvT, wpT, yT)
        else:
            _body(nc, tc, xT, wqT, wkT, wvT, wpT, yT)
    nc.compile()
    return nc


def _body(nc, tc, xT, wqT, wkT, wvT, wpT, yT):
    with (
        tc.tile_pool(name="big", bufs=1) as big,
        tc.tile_pool(name="wsl", bufs=2) as wsl,
        tc.tile_pool(name="qk", bufs=2) as qkp,
        tc.tile_pool(name="pfull", bufs=4 if OPTS.get("lead3") else 3) as pfp,
        tc.tile_pool(name="pband", bufs=1) as pbp,
        tc.tile_pool(name="small", bufs=2) as sp,
        tc.tile_pool(name="small1", bufs=1) as sp1,
        tc.tile_pool(name="ost", bufs=6) as ostp,
        tc.tile_pool(name="osb", bufs=2) as osbp,
        tc.tile_pool(name="psA", bufs=2, space="PSUM") as psA,
        tc.tile_pool(name="psP", bufs=2, space="PSUM") as psP,
        tc.tile_pool(name="psO", bufs=2, space="PSUM") as psO,
    ):
        # ---- persistent loads -------------------------------------------
        # xT split into seq-chunks so compute can start before the full load;
        # wv + first chunks lead so the V projection starts ~8us in
        xT_sb = big.tile([P, DT, S], BF16, tag="xT")
        xT_src = xT.ap().rearrange("(o p) s -> p o s", p=P)
        xw = S // XCH

        def load_xchunk(c):
            nc.sync.dma_start(
                xT_sb[:, :, c * xw:(c + 1) * xw], xT_src[:, :, c * xw:(c + 1) * xw])

        wvT_sb = big.tile([P, DT, 512], BF16, tag="wv")
        wv_src = wvT.ap().rearrange("(o p) e -> p o e", p=P)
        # per-group Q/K weight slices, double-buffered + prefetched
        wq_sb = [None] * NG
        wk_sb = [None] * NG

        def load_wqk(g):
            wq_sb[g] = wsl.tile([P, DT, P], BF16, tag="wq", name=f"wq_{g}")
            nc.sync.dma_start(
                wq_sb[g],
                wqT.ap().rearrange("(o p) e -> p o e", p=P)[:, :, g * P:(g + 1) * P],
            )
            wk_sb[g] = wsl.tile([P, DT, P], BF16, tag="wk", name=f"wk_{g}")
            nc.sync.dma_start(
                wk_sb[g],
                wkT.ap().rearrange("(o p) e -> p o e", p=P)[:, :, g * P:(g + 1) * P],
            )

        if OPTS.get("starter"):
            with tc.high_priority():
                load_wqk(0)
                load_xchunk(0)
                load_xchunk(1)
        if not OPTS.get("no_dma_split"):
            # wv on the Activation HWDGE queue, x chunks on SP: parallel
            nc.scalar.dma_start(wvT_sb, wv_src)
        else:
            nc.sync.dma_start(wvT_sb, wv_src)
        if not OPTS.get("starter"):
            load_xchunk(0)
            load_xchunk(1)
            load_wqk(0)
        for c in range(2, XCH):
            load_xchunk(c)
        load_wqk(1)

        wpT_sb = big.tile([P, 4, D], BF16, tag="wpT")

        # V with a ones column per head: [P, kt, 8 heads * 65]
        vaug = big.tile([P, KT, NHC * 65], F32R, tag="vaug")
        ones_cols = vaug.rearrange("p t (h c) -> p t h c", c=65)[:, :, :, 64]
        nc.gpsimd.memset(ones_cols.bitcast(F32), 1.0)

        # warm the ACT exp table while DMAs are in flight (first call to a
        # new activation set costs ~2.7us; keep it off the attention path)
        zwarm = sp1.tile([1, 1], F32, tag="zwarm")
        if not OPTS.get("no_dma_split"):
            nc.scalar.activation(zwarm, xT_sb[0:1, 0, 0:1], EXP, scale=SCALE)
        else:
            nc.scalar.activation(zwarm, wvT_sb[0:1, 0, 0:1], EXP, scale=SCALE)

        # ---- starter: Q/K g0 window 0 in 256-col halves — the first
        # compute, gated only on wq0/wk0 + x chunks 0/1 (not the 1MB wv)
        qT = [None] * NG
        kT = [None] * NG
        if OPTS.get("starter"):
            for which in ("q", "k"):
                dst = qkp.tile([P, S], BF16, tag=which, name=f"{which}T_0")
                if which == "q":
                    qT[0] = dst
                else:
                    kT[0] = dst
            pj0 = {which: psP.tile([P, QW], F32, tag="pj",
                                   name=f"pj0_{which}")
                   for which in ("q", "k")}
            for hh in range(2):
                for which in ("q", "k"):
                    w_sb = wq_sb[0] if which == "q" else wk_sb[0]
                    for dk in range(DT):
                        nc.tensor.matmul(
                            pj0[which][:, hh * 256:(hh + 1) * 256],
                            lhsT=w_sb[:, dk, :],
                            rhs=xT_sb[:, dk, hh * 256:(hh + 1) * 256],
                            start=(dk == 0), stop=(dk == DT - 1),
                        )
                    if hh == 1:
                        dst = qT[0] if which == "q" else kT[0]
                        nc.vector.tensor_copy(
                            out=dst[:, 0:QW], in_=pj0[which])

        # ---- V projection (all 8 heads, one seq-chunk per psum tile) -----
        def emit_vproj_chunk(sp2):
            pv = psA.tile([P, 2, QW], F32, tag="mm", name=f"pv_{sp2}")
            for half in range(2):
                st = 2 * sp2 + half
                for dk in range(DT):
                    nc.tensor.matmul(
                        pv[:, half, :],
                        lhsT=xT_sb[:, dk, st * P:(st + 1) * P],
                        rhs=wvT_sb[:, dk, :],
                        start=(dk == 0), stop=(dk == DT - 1),
                    )
            nc.vector.tensor_copy(
                out=vaug[:, 2 * sp2:2 * sp2 + 2, :]
                    .rearrange("p t (h c) -> p t h c", c=65)[:, :, :, 0:64],
                in_=pv.rearrange("p t (h c) -> p t h c", c=64),
            )

        for sp2 in range(4):
            emit_vproj_chunk(sp2)

        # wp DMA late (after the x/wv/wq/wk burst; needed only at the end)
        nc.sync.dma_start(wpT_sb, wpT.ap().rearrange("(t p) e -> p t e", p=P))

        # output accumulator O'[do, q] (do = local_head*64 + hd), normalized
        oall = big.tile([P, NG, S], BF16, tag="oall")

        # ---- PE filler queue: single-matmul closures (next group's Q/K
        # proj, g3's output proj) drained into ACT-paced attention bubbles;
        # the PE issues in-order, so later-emitted work cannot overtake a
        # stalled AV unless we zipper it in at emission time.
        filler = []
        drained = set()

        def _pop1():
            lb, fn = filler.pop(0)
            fn()
            if lb is not None:
                drained.add(lb)

        def fill(n):
            for _ in range(min(n, len(filler))):
                _pop1()

        def fill_all():
            while filler:
                _pop1()

        def drain_until(label):
            if label in drained:
                return
            while filler:
                lb, fn = filler.pop(0)
                fn()
                if lb is not None:
                    drained.add(lb)
                if lb == label:
                    return

        def push_proj_sub(g, which, half):
            """Queue one 1024-seq half of group g's Q or K projection."""
            w_sb = wq_sb[g] if which == "q" else wk_sb[g]
            if half == 0 and (qT[g] if which == "q" else kT[g]) is None:
                dst = qkp.tile([P, S], BF16, tag=which, name=f"{which}T_{g}")
                if which == "q":
                    qT[g] = dst
                else:
                    kT[g] = dst
            dst = qT[g] if which == "q" else kT[g]
            if OPTS.get("qk_pair"):
                # window-pair interleave: [sw_a dk, sw_b dk] so consecutive
                # matmuls share lhsT (halves LDWEIGHTS on HW)
                boxes = [{}, {}]

                def mmp(dk, j, boxes=boxes, half=half):
                    sw = 2 * half + j
                    box = boxes[j]
                    if dk == 0:
                        box["pt"] = psP.tile(
                            [P, QW], F32, tag="pj",
                            name=f"pj_{which}_{g}_{sw}")
                    nc.tensor.matmul(
                        box["pt"],
                        lhsT=w_sb[:, dk, :],
                        rhs=xT_sb[:, dk, sw * QW:(sw + 1) * QW],
                        start=(dk == 0), stop=(dk == DT - 1),
                    )
                    if dk == DT - 1:
                        nc.vector.tensor_copy(
                            out=dst[:, sw * QW:(sw + 1) * QW], in_=box["pt"])

                for dk in range(DT):
                    for j in range(2):
                        filler.append(
                            (None, lambda dk=dk, j=j: mmp(dk, j)))
                return
            # g0's sw0 is done by the starter
            jlo = 1 if (g == 0 and half == 0
                        and OPTS.get("starter")) else 0
            for j in range(jlo, 2):
                sw = 2 * half + j
                box = {}

                def mm(dk, sw=sw, box=box):
                    if dk == 0:
                        box["pt"] = psP.tile(
                            [P, QW], F32, tag="pj",
                            name=f"pj_{which}_{g}_{sw}")
                    nc.tensor.matmul(
                        box["pt"],
                        lhsT=w_sb[:, dk, :],
                        rhs=xT_sb[:, dk, sw * QW:(sw + 1) * QW],
                        start=(dk == 0), stop=(dk == DT - 1),
                    )
                    if dk == DT - 1:
                        nc.vector.tensor_copy(
                            out=dst[:, sw * QW:(sw + 1) * QW], in_=box["pt"])

                for dk in range(DT):
                    lb = (f"proj_{g}_{which}_{half}"
                          if (j == 1 and dk == DT - 1) else None)
                    filler.append((lb, lambda dk=dk, mm=mm: mm(dk)))

        def push_outproj_sw(sw):
            # yT[e, sw-window] = sum_t wpT[:, t, e].T @ oall[:, t, sw-window]
            for eb in range(D // P):
                box = {}

                def mm(t, eb=eb, box=box):
                    if t == 0:
                        box["pt"] = psP.tile(
                            [P, QW], F32, tag="pj", name=f"pfin_{sw}_{eb}")
                    nc.tensor.matmul(
                        box["pt"],
                        lhsT=wpT_sb[:, t, eb * P:(eb + 1) * P],
                        rhs=oall[:, t, sw * QW:(sw + 1) * QW],
                        start=(t == 0), stop=(t == NG - 1),
                    )
                    if t == NG - 1:
                        ot = ostp.tile([P, QW], BF16, tag="ot",
                                       name=f"ot_{sw}_{eb}")
                        nc.vector.tensor_copy(out=ot, in_=box["pt"])
                        nc.sync.dma_start(
                            yT.ap()[eb * P:(eb + 1) * P,
                                    sw * QW:(sw + 1) * QW],
                            ot,
                        )

                for t in range(NG):
                    filler.append((None, lambda t=t, mm=mm: mm(t)))

        # group 0 Q/K projection emitted straight; V chunks 4-7 become
        # filler for attention(g0) (qb0/qb1 only touch k-tiles 0-7 = chunks
        # 0-3; drain_until guards qb2/qb3)
        for which in ("q", "k"):
            for half in range(2):
                push_proj_sub(0, which, half)
        fill_all()
        # emit V chunks 4-7 straight: as filler inside g0's band block their
        # psA allocs would rotate into the band-scores slots and stall on
        # the same exp chain the first AV waits for
        for sp2 in range(4, KT // 2):
            emit_vproj_chunk(sp2)

        # ---- per head-pair-group: attention + zippered filler -----------
        for g in range(NG):
            if g + 2 <= NG - 1:
                load_wqk(g + 2)
            if g + 1 <= NG - 1:
                for which in ("q", "k"):
                    for half in range(2):
                        push_proj_sub(g + 1, which, half)
            qTg, kTg = qT[g], kT[g]

            qb_order = (range(QB) if (g + 1 <= NG - 1 or not OPTS.get('g3_rev'))
                        else range(QB - 1, -1, -1))
            for qb in qb_order:
                # group g's own proj may still sit in the filler queue
                # (held back across the group boundary as early-qb filler);
                # force-drain exactly what this qb's scores need
                if qb <= 1:
                    drain_until(f"proj_{g}_q_0")
                    drain_until(f"proj_{g}_k_0")
                else:
                    drain_until(f"proj_{g}_q_1")
                    drain_until(f"proj_{g}_k_1")
                nfull = 4 * qb  # full k-tiles 0 .. 4qb-1, then 4 band tiles
                po = [
                    psO.tile([65, QW], F32, tag="po", name=f"po_{g}_{qb}_{hl}")
                    for hl in range(2)
                ]
                pband = pbp.tile([P, 2, 4, QW], F32R, tag="pband",
                                 name=f"pband_{g}_{qb}")

                def scores_full(kt, qb=qb, qTg=qTg, kTg=kTg, g=g):
                    ps2 = psA.tile([P, 2, QW], F32, tag="mm",
                                   name=f"ps_{g}_{qb}_{kt}")
                    for hl in range(2):
                        hp = hl * 64
                        nc.tensor.matmul(
                            ps2[:, hl, :],
                            lhsT=kTg[hp:hp + 64, kt * P:(kt + 1) * P],
                            rhs=qTg[hp:hp + 64, qb * QW:(qb + 1) * QW],
                            start=True, stop=True,
                            tile_position=(0, 0)
                            if OPTS.get("serial_scores") else None,
                        )
                    pp = pfp.tile([P, 2, QW], F32R, tag="pf",
                                  name=f"pf_{g}_{qb}_{kt}")
                    nc.scalar.activation(pp, ps2, EXP, scale=SCALE)
                    return pp

                # 1) band scores + exp + per-rel mask first (their ACT/gpsimd
                #    chain overlaps filler + leading full scores)
                for rel in range(4):
                    kt = 4 * qb + rel
                    v0 = P * rel
                    ps2 = psA.tile([P, 2, QW], F32, tag="mm",
                                   name=f"ps_{g}_{qb}_{kt}")
                    for hl in range(2):
                        hp = hl * 64
                        nc.tensor.matmul(
                            ps2[:, hl, v0:],
                            lhsT=kTg[hp:hp + 64, kt * P:(kt + 1) * P],
                            rhs=qTg[hp:hp + 64, qb * QW + v0:(qb + 1) * QW],
                            start=True, stop=True,
                            tile_position=(0, 0)
                            if OPTS.get("serial_scores") else None,
                        )
                    w0 = min(v0, QW - 2 * P)  # AV reads from here
                    w1 = QW if rel == 3 else min(v0 + P, QW)
                    if OPTS.get("band_split"):
                        # per-hl exp + mask: first band AV (hl0) waits only
                        # half the ACT/Pool chain
                        for hl in range(2):
                            nc.scalar.activation(
                                pband[:, hl, rel, v0:], ps2[:, hl, v0:],
                                EXP, scale=SCALE)
                            nc.gpsimd.affine_select(
                                out=pband[:, hl, rel, w0:w1],
                                in_=pband[:, hl, rel, w0:w1],
                                compare_op=mybir.AluOpType.is_ge, fill=0.0,
                                base=w0 - P * rel, channel_multiplier=-1,
                                pattern=[[0, 1], [1, w1 - w0]],
                            )
                    else:
                        nc.scalar.activation(
                            pband[:, :, rel, v0:], ps2[:, :, v0:], EXP,
                            scale=SCALE)
                        nc.gpsimd.affine_select(
                            out=pband[:, :, rel, w0:w1],
                            in_=pband[:, :, rel, w0:w1],
                            compare_op=mybir.AluOpType.is_ge, fill=0.0,
                            base=w0 - P * rel, channel_multiplier=-1,
                            pattern=[[0, 2], [1, w1 - w0]],
                        )
                    fill(1)

                # 2) leading full scores + filler bridge the band-exp chain
                lead = min(3 if OPTS.get("lead3") else 2, nfull)
                pps = {kt: scores_full(kt) for kt in range(lead)}
                fill(4)

                # 3) band AVs (rel0 covers all columns => carries start)
                for rel in range(4):
                    kt = 4 * qb + rel
                    av0 = min(P * rel, QW - 2 * P)
                    for hl in range(2):
                        h = 2 * g + hl
                        nc.tensor.matmul(
                            po[hl][:, av0:],
                            lhsT=vaug[:, kt, h * 65:(h + 1) * 65],
                            rhs=pband[:, hl, rel, av0:],
                            start=(rel == 0), stop=(rel == 3 and nfull == 0),
                        )
                    fill(1)

                # 4) remaining full tiles, scores->exp->AV with 1-filler gaps
                for kt in range(nfull):
                    if kt not in pps:
                        pps[kt] = scores_full(kt)
                    if kt + lead < nfull:
                        pps[kt + lead] = scores_full(kt + lead)
                    fill(1)
                    pp = pps.pop(kt)
                    for hl in range(2):
                        h = 2 * g + hl
                        nc.tensor.matmul(
                            po[hl],
                            lhsT=vaug[:, kt, h * 65:(h + 1) * 65],
                            rhs=pp[:, hl, :],
                            start=False, stop=(kt == nfull - 1),
                        )

                # copy po -> SBUF immediately so the PSUM buf frees after one
                # fast DVE op; the recip/bcast/mul chain then runs from SBUF
                # without stalling the next qb's AV (psO buf wait)
                ocp = osbp.tile([65, 2, QW], F32, tag="ocp",
                                name=f"ocp_{g}_{qb}")
                for hl in range(2):
                    nc.vector.tensor_copy(out=ocp[:, hl, :], in_=po[hl])
                for hl in range(2):
                    zrow = sp1.tile([1, QW], F32, tag="zrow",
                                    name=f"zr_{g}_{qb}_{hl}")
                    nc.vector.tensor_copy(out=zrow, in_=ocp[64:65, hl, :])
                    recip = sp1.tile([1, QW], F32, tag="recip",
                                     name=f"rc_{g}_{qb}_{hl}")
                    nc.vector.reciprocal_approx_fast(recip, zrow)
                    bc = sp.tile([64, QW], F32, tag="bc",
                                 name=f"bc_{g}_{qb}_{hl}")
                    nc.gpsimd.partition_broadcast(bc, recip)
                    nc.vector.tensor_mul(
                        out=oall[hl * 64:(hl + 1) * 64, g, qb * QW:(qb + 1) * QW],
                        in0=ocp[0:64, hl, :],
                        in1=bc,
                    )

                if g + 1 > NG - 1:
                    # g3: this qb's s-window of the output projection becomes
                    # filler for the remaining qbs
                    push_outproj_sw(qb)

            # hold remaining filler across the group boundary: the next
            # group's early qbs (esp. their band chains) are filler-starved;
            # drain_until guards enforce the proj deps each qb needs
            if g == NG - 1 or OPTS.get("no_holdback"):
                fill_all()

def _get_nc():
    global _NC
    if _NC is None:
        _NC = _build()
    return _NC


def _in_maps(x, w_qkv, w_proj):
    from ml_dtypes import bfloat16

    x = np.asarray(x, dtype=np.float32)
    w_qkv = np.asarray(w_qkv, dtype=np.float32)
    w_proj = np.asarray(w_proj, dtype=np.float32)

    maps = []
    for c in range(8):
        b, hh = c // 2, c % 2
        lo, hi = hh * 512, (hh + 1) * 512
        maps.append({
            "xT": np.ascontiguousarray(x[b].T).astype(bfloat16),
            "wqT": np.ascontiguousarray(w_qkv[lo:hi].T).astype(bfloat16),
            "wkT": np.ascontiguousarray(w_qkv[D + lo:D + hi].T).astype(bfloat16),
            "wvT": np.ascontiguousarray(w_qkv[2 * D + lo:2 * D + hi].T).astype(bfloat16),
            "wpT": np.ascontiguousarray(w_proj[:, lo:hi].T).astype(bfloat16),
        })
    return maps


def kernel(x, w_qkv, w_proj):
    from concourse.bass_utils import run_bass_kernel_spmd

    in_maps = _in_maps(x, w_qkv, w_proj)
    res = run_bass_kernel_spmd(_get_nc(), in_maps, core_ids=list(range(8)))
    out = np.empty((B, S, D), dtype=np.float32)
    for b in range(B):
        out[b] = (res.results[2 * b]["yT"].astype(np.float32)
                  + res.results[2 * b + 1]["yT"].astype(np.float32)).T
    return out



# revision 48
# speedup vs baseline: 1.0946x; 1.0066x over previous
"""Causal self-attention kernel for 8 Trainium2 NeuronCores.

Problem: B=4, S=2048, D=1024, H=16, HD=64 (fp32).
  qkv = x @ w_qkv.T ; per-head causal softmax attention ; out @ w_proj.T

Sharding: core c handles batch b = c//2 and head-half hh = c%2 (8 heads).
Each core computes its 8 heads' attention and a partial output projection
(w_proj column slice); the host sums the two partials per batch.

v1 pipeline (vs baseline):
  - all weight DMAs prefetched (2-buf group slices); no mid-kernel PE
    stalls on DMA -> HAM clock gate stays warm (2.4 GHz)
  - Q/K proj with weights stationary (dk-outer, s-windows moving)
  - group g+1 Q/K proj matmuls emission-interleaved into attention(g)'s
    qb loop so the in-order PE fills exp(ACT)-paced gaps
  - epilogue uses reciprocal_approx_fast (DVE custom op, ~5x)
  - V/P/attention-band in bf16 (AV matmuls bf16; PE rate unchanged)
  - out proj w-stationary in bf16 -> yT [e, s]; host transposes
  - PSUM: psA bufs=2 (4 banks) + proj pool (2) + psO (2) = 8

v2 (HW 294us single-shot -> ~269us loop steady-state):
  - copy-first epilogue: po (psum) -> ocp (SBUF) immediately after the AV
    stop so the psO buf frees after one DVE copy; recip/bcast/mul then run
    from SBUF off the next qb's AV critical path (the psO pair-per-qb
    rotation was costing 400-800ns per qb)
  - g3 processes qbs FORWARD so the big qb3 (most fill slots) overlaps the
    bulk of the outproj; the tail is only sw3's outproj
  - start2: wq0/wk0 + x chunks 0/1 lead the serial DMA transfer order and
    a Q/K g0-sw0 "starter" (256-col halves) is emitted as the first
    compute; wv transfers behind them on SP
  - scores pairs run row-tiled concurrent on HW (auto tile_position from
    base partitions 0/64; measured 336ns/pair vs 947 serial-forced)
  - measured dead ends (microbenched): fp8 DoubleRow scores are
    LDWEIGHTS-bound (559ns/pair, worse than bf16 336); col-tiled M=64 AV
    pairs are 1.64x faster (337 vs 553) but every scheme to recover the
    softmax denominator (z) off the ones-row costs back the gain (quad
    M=1 ones-matmuls: +212ns/pair; DVE partition-tree: +50-85us DVE);
    outproj phase-split into per-t passes trades psum accumulation for
    +74us of DVE copies
"""

import sys

if "/opt/trn_rl_repo" not in sys.path:
    sys.path.insert(0, "/opt/trn_rl_repo")

import numpy as np

import concourse.tile as tile
from concourse import bacc, mybir

F32 = mybir.dt.float32
F32R = mybir.dt.float32r
BF16 = mybir.dt.bfloat16
EXP = mybir.ActivationFunctionType.Exp

B, S, D = 4, 2048, 1024
H, HD = 16, 64
P = 128
DT = D // P            # 8 d-tiles (contraction tiles for projections)
NHC = 8                # heads per core
NG = NHC // 2          # head pair-groups per core
QB = 4                 # q-blocks of 512
QW = 512               # q-block width
KT = S // P            # 16 k-tiles
XCH = 8                # xT DMA split chunks (along seq)
SCALE = 1.0 / np.sqrt(HD)

_NC = None

# experiment flags (set via _build(opts=...)); default {} = baseline
OPTS = {}


def _build(loop_reps=1, opts=None):
    global OPTS
    OPTS = opts or {}
    nc = bacc.Bacc("TRN2", target_bir_lowering=False, debug=False)

    xT = nc.dram_tensor("xT", [D, S], BF16, kind="ExternalInput")
    wqT = nc.dram_tensor("wqT", [D, 512], BF16, kind="ExternalInput")
    wkT = nc.dram_tensor("wkT", [D, 512], BF16, kind="ExternalInput")
    wvT = nc.dram_tensor("wvT", [D, 512], BF16, kind="ExternalInput")
    wpT = nc.dram_tensor("wpT", [512, D], BF16, kind="ExternalInput")
    yT = nc.dram_tensor("yT", [D, S], BF16, kind="ExternalOutput")

    with tile.TileContext(nc) as tc:
        if loop_reps > 1:
            with tc.For_i(0, loop_reps, 1):
                _body(nc, tc, xT, wqT, wkT, wvT, wpT, yT)
        else:
            _body(nc, tc, xT, wqT, wkT, wvT, wpT, yT)
    nc.compile()
    return nc


def _body(nc, tc, xT, wqT, wkT, wvT, wpT, yT):
    with (
        tc.tile_pool(name="big", bufs=1) as big,
        tc.tile_pool(name="wsl", bufs=2) as wsl,
        tc.tile_pool(name="qk", bufs=2) as qkp,
        tc.tile_pool(name="pfull", bufs=4 if OPTS.get("lead3") else 3) as pfp,
        tc.tile_pool(name="pband", bufs=1) as pbp,
        tc.tile_pool(name="small", bufs=2) as sp,
        tc.tile_pool(name="small1", bufs=1) as sp1,
        tc.tile_pool(name="ost", bufs=6) as ostp,
        tc.tile_pool(name="osb", bufs=2) as osbp,
        tc.tile_pool(name="psA", bufs=2, space="PSUM") as psA,
        tc.tile_pool(name="psP", bufs=2, space="PSUM") as psP,
        tc.tile_pool(name="psO", bufs=2, space="PSUM") as psO,
    ):
        # ---- persistent loads -------------------------------------------
        # xT split into seq-chunks so compute can start before the full load;
        # wv + first chunks lead so the V projection starts ~8us in
        xT_sb = big.tile([P, DT, S], BF16, tag="xT")
        xT_src = xT.ap().rearrange("(o p) s -> p o s", p=P)
        xw = S // XCH

        def load_xchunk(c):
            nc.sync.dma_start(
                xT_sb[:, :, c * xw:(c + 1) * xw], xT_src[:, :, c * xw:(c + 1) * xw])

        wvT_sb = big.tile([P, DT, 512], BF16, tag="wv")
        wv_src = wvT.ap().rearrange("(o p) e -> p o e", p=P)
        # per-group Q/K weight slices, double-buffered + prefetched
        wq_sb = [None] * NG
        wk_sb = [None] * NG

        def load_wqk(g):
            wq_sb[g] = wsl.tile([P, DT, P], BF16, tag="wq", name=f"wq_{g}")
            nc.sync.dma_start(
                wq_sb[g],
                wqT.ap().rearrange("(o p) e -> p o e", p=P)[:, :, g * P:(g + 1) * P],
            )
            wk_sb[g] = wsl.tile([P, DT, P], BF16, tag="wk", name=f"wk_{g}")
            nc.sync.dma_start(
                wk_sb[g],
                wkT.ap().rearrange("(o p) e -> p o e", p=P)[:, :, g * P:(g + 1) * P],
            )

        if not OPTS.get("no_start2"):
            # starter deps first in the serial transfer order; wv (only
            # needed by V proj, which runs after the starter) behind them
            load_wqk(0)
            load_xchunk(0)
            load_xchunk(1)
            nc.sync.dma_start(wvT_sb, wv_src)
        else:
            if not OPTS.get("no_dma_split"):
                # wv on the Activation HWDGE queue, x chunks on SP
                nc.scalar.dma_start(wvT_sb, wv_src)
            else:
                nc.sync.dma_start(wvT_sb, wv_src)
            load_xchunk(0)
            load_xchunk(1)
            load_wqk(0)
        for c in range(2, XCH):
            load_xchunk(c)
        load_wqk(1)

        wpT_sb = big.tile([P, 4, D], BF16, tag="wpT")

        # V with a ones column per head: [P, kt, 8 heads * 65]
        vaug = big.tile([P, KT, NHC * 65], F32R, tag="vaug")
        ones_cols = vaug.rearrange("p t (h c) -> p t h c", c=65)[:, :, :, 64]
        nc.gpsimd.memset(ones_cols.bitcast(F32), 1.0)

        # warm the ACT exp table while DMAs are in flight (first call to a
        # new activation set costs ~2.7us; keep it off the attention path)
        zwarm = sp1.tile([1, 1], F32, tag="zwarm")
        if not OPTS.get("no_dma_split"):
            nc.scalar.activation(zwarm, xT_sb[0:1, 0, 0:1], EXP, scale=SCALE)
        else:
            nc.scalar.activation(zwarm, wvT_sb[0:1, 0, 0:1], EXP, scale=SCALE)

        # ---- starter: Q/K g0 window 0 in 256-col halves — the first
        # compute, gated only on wq0/wk0 + x chunks 0/1 (not the 1MB wv)
        qT = [None] * NG
        kT = [None] * NG
        if OPTS.get("starter") or not OPTS.get("no_start2"):
            for which in ("q", "k"):
                dst = qkp.tile([P, S], BF16, tag=which, name=f"{which}T_0")
                if which == "q":
                    qT[0] = dst
                else:
                    kT[0] = dst
            pj0 = {which: psP.tile([P, QW], F32, tag="pj",
                                   name=f"pj0_{which}")
                   for which in ("q", "k")}
            for hh in range(2):
                for which in ("q", "k"):
                    w_sb = wq_sb[0] if which == "q" else wk_sb[0]
                    for dk in range(DT):
                        nc.tensor.matmul(
                            pj0[which][:, hh * 256:(hh + 1) * 256],
                            lhsT=w_sb[:, dk, :],
                            rhs=xT_sb[:, dk, hh * 256:(hh + 1) * 256],
                            start=(dk == 0), stop=(dk == DT - 1),
                        )
                    if hh == 1:
                        dst = qT[0] if which == "q" else kT[0]
                        nc.vector.tensor_copy(
                            out=dst[:, 0:QW], in_=pj0[which])

        # ---- V projection (all 8 heads, one seq-chunk per psum tile) -----
        def emit_vproj_chunk(sp2):
            pv = psA.tile([P, 2, QW], F32, tag="mm", name=f"pv_{sp2}")
            for half in range(2):
                st = 2 * sp2 + half
                for dk in range(DT):
                    nc.tensor.matmul(
                        pv[:, half, :],
                        lhsT=xT_sb[:, dk, st * P:(st + 1) * P],
                        rhs=wvT_sb[:, dk, :],
                        start=(dk == 0), stop=(dk == DT - 1),
                    )
            nc.vector.tensor_copy(
                out=vaug[:, 2 * sp2:2 * sp2 + 2, :]
                    .rearrange("p t (h c) -> p t h c", c=65)[:, :, :, 0:64],
                in_=pv.rearrange("p t (h c) -> p t h c", c=64),
            )

        for sp2 in range(4):
            emit_vproj_chunk(sp2)

        # wp DMA late (after the x/wv/wq/wk burst; needed only at the end)
        nc.sync.dma_start(wpT_sb, wpT.ap().rearrange("(t p) e -> p t e", p=P))

        # output accumulator O'[do, q] (do = local_head*64 + hd), normalized
        oall = big.tile([P, NG, S], BF16, tag="oall")

        # ---- PE filler queue: single-matmul closures (next group's Q/K
        # proj, g3's output proj) drained into ACT-paced attention bubbles;
        # the PE issues in-order, so later-emitted work cannot overtake a
        # stalled AV unless we zipper it in at emission time.
        filler = []
        drained = set()

        def _pop1():
            lb, fn = filler.pop(0)
            fn()
            if lb is not None:
                drained.add(lb)

        def fill(n):
            for _ in range(min(n, len(filler))):
                _pop1()

        def fill_all():
            while filler:
                _pop1()

        def drain_until(label):
            if label in drained:
                return
            while filler:
                lb, fn = filler.pop(0)
                fn()
                if lb is not None:
                    drained.add(lb)
                if lb == label:
                    return

        def push_proj_sub(g, which, half):
            """Queue one 1024-seq half of group g's Q or K projection."""
            w_sb = wq_sb[g] if which == "q" else wk_sb[g]
            if half == 0 and (qT[g] if which == "q" else kT[g]) is None:
                dst = qkp.tile([P, S], BF16, tag=which, name=f"{which}T_{g}")
                if which == "q":
                    qT[g] = dst
                else:
                    kT[g] = dst
            dst = qT[g] if which == "q" else kT[g]
            if OPTS.get("qk_pair"):
                # window-pair interleave: [sw_a dk, sw_b dk] so consecutive
                # matmuls share lhsT (halves LDWEIGHTS on HW)
                boxes = [{}, {}]

                def mmp(dk, j, boxes=boxes, half=half):
                    sw = 2 * half + j
                    box = boxes[j]
                    if dk == 0:
                        box["pt"] = psP.tile(
                            [P, QW], F32, tag="pj",
                            name=f"pj_{which}_{g}_{sw}")
                    nc.tensor.matmul(
                        box["pt"],
                        lhsT=w_sb[:, dk, :],
                        rhs=xT_sb[:, dk, sw * QW:(sw + 1) * QW],
                        start=(dk == 0), stop=(dk == DT - 1),
                    )
                    if dk == DT - 1:
                        nc.vector.tensor_copy(
                            out=dst[:, sw * QW:(sw + 1) * QW], in_=box["pt"])

                for dk in range(DT):
                    for j in range(2):
                        filler.append(
                            (None, lambda dk=dk, j=j: mmp(dk, j)))
                return
            # g0's sw0 is done by the starter
            jlo = 1 if (g == 0 and half == 0
                        and (OPTS.get("starter")
                             or not OPTS.get("no_start2"))) else 0
            for j in range(jlo, 2):
                sw = 2 * half + j
                box = {}

                def mm(dk, sw=sw, box=box):
                    if dk == 0:
                        box["pt"] = psP.tile(
                            [P, QW], F32, tag="pj",
                            name=f"pj_{which}_{g}_{sw}")
                    nc.tensor.matmul(
                        box["pt"],
                        lhsT=w_sb[:, dk, :],
                        rhs=xT_sb[:, dk, sw * QW:(sw + 1) * QW],
                        start=(dk == 0), stop=(dk == DT - 1),
                    )
                    if dk == DT - 1:
                        nc.vector.tensor_copy(
                            out=dst[:, sw * QW:(sw + 1) * QW], in_=box["pt"])

                for dk in range(DT):
                    lb = (f"proj_{g}_{which}_{half}"
                          if (j == 1 and dk == DT - 1) else None)
                    filler.append((lb, lambda dk=dk, mm=mm: mm(dk)))

        def push_outproj_sw(sw):
            # yT[e, sw-window] = sum_t wpT[:, t, e].T @ oall[:, t, sw-window]
            for eb in range(D // P):
                box = {}

                def mm(t, eb=eb, box=box):
                    if t == 0:
                        box["pt"] = psP.tile(
                            [P, QW], F32, tag="pj", name=f"pfin_{sw}_{eb}")
                    nc.tensor.matmul(
                        box["pt"],
                        lhsT=wpT_sb[:, t, eb * P:(eb + 1) * P],
                        rhs=oall[:, t, sw * QW:(sw + 1) * QW],
                        start=(t == 0), stop=(t == NG - 1),
                    )
                    if t == NG - 1:
                        ot = ostp.tile([P, QW], BF16, tag="ot",
                                       name=f"ot_{sw}_{eb}")
                        nc.vector.tensor_copy(out=ot, in_=box["pt"])
                        nc.sync.dma_start(
                            yT.ap()[eb * P:(eb + 1) * P,
                                    sw * QW:(sw + 1) * QW],
                            ot,
                        )

                for t in range(NG):
                    filler.append((None, lambda t=t, mm=mm: mm(t)))

        # group 0 Q/K projection emitted straight; V chunks 4-7 become
        # filler for attention(g0) (qb0/qb1 only touch k-tiles 0-7 = chunks
        # 0-3; drain_until guards qb2/qb3)
        for which in ("q", "k"):
            for half in range(2):
                push_proj_sub(0, which, half)
        fill_all()
        # emit V chunks 4-7 straight: as filler inside g0's band block their
        # psA allocs would rotate into the band-scores slots and stall on
        # the same exp chain the first AV waits for
        for sp2 in range(4, KT // 2):
            emit_vproj_chunk(sp2)

        # ---- per head-pair-group: attention + zippered filler -----------
        for g in range(NG):
            if g + 2 <= NG - 1:
                load_wqk(g + 2)
            if g + 1 <= NG - 1:
                for which in ("q", "k"):
                    for half in range(2):
                        push_proj_sub(g + 1, which, half)
            qTg, kTg = qT[g], kT[g]

            qb_order = (range(QB) if (g + 1 <= NG - 1 or not OPTS.get('g3_rev'))
                        else range(QB - 1, -1, -1))
            for qb in qb_order:
                # group g's own proj may still sit in the filler queue
                # (held back across the group boundary as early-qb filler);
                # force-drain exactly what this qb's scores need
                if qb <= 1:
                    drain_until(f"proj_{g}_q_0")
                    drain_until(f"proj_{g}_k_0")
                else:
                    drain_until(f"proj_{g}_q_1")
                    drain_until(f"proj_{g}_k_1")
                nfull = 4 * qb  # full k-tiles 0 .. 4qb-1, then 4 band tiles
                po = [
                    psO.tile([65, QW], F32, tag="po", name=f"po_{g}_{qb}_{hl}")
                    for hl in range(2)
                ]
                pband = pbp.tile([P, 2, 4, QW], F32R, tag="pband",
                                 name=f"pband_{g}_{qb}")

                def scores_full(kt, qb=qb, qTg=qTg, kTg=kTg, g=g):
                    ps2 = psA.tile([P, 2, QW], F32, tag="mm",
                                   name=f"ps_{g}_{qb}_{kt}")
                    for hl in range(2):
                        hp = hl * 64
                        nc.tensor.matmul(
                            ps2[:, hl, :],
                            lhsT=kTg[hp:hp + 64, kt * P:(kt + 1) * P],
                            rhs=qTg[hp:hp + 64, qb * QW:(qb + 1) * QW],
                            start=True, stop=True,
                            tile_position=(0, 0)
                            if OPTS.get("serial_scores") else None,
                        )
                    pp = pfp.tile([P, 2, QW], F32R, tag="pf",
                                  name=f"pf_{g}_{qb}_{kt}")
                    nc.scalar.activation(pp, ps2, EXP, scale=SCALE)
                    return pp

                # 1) band scores + exp + per-rel mask first (their ACT/gpsimd
                #    chain overlaps filler + leading full scores)
                for rel in range(4):
                    kt = 4 * qb + rel
                    v0 = P * rel
                    ps2 = psA.tile([P, 2, QW], F32, tag="mm",
                                   name=f"ps_{g}_{qb}_{kt}")
                    for hl in range(2):
                        hp = hl * 64
                        nc.tensor.matmul(
                            ps2[:, hl, v0:],
                            lhsT=kTg[hp:hp + 64, kt * P:(kt + 1) * P],
                            rhs=qTg[hp:hp + 64, qb * QW + v0:(qb + 1) * QW],
                            start=True, stop=True,
                            tile_position=(0, 0)
                            if OPTS.get("serial_scores") else None,
                        )
                    w0 = min(v0, QW - 2 * P)  # AV reads from here
                    w1 = QW if rel == 3 else min(v0 + P, QW)
                    if OPTS.get("band_split"):
                        # per-hl exp + mask: first band AV (hl0) waits only
                        # half the ACT/Pool chain
                        for hl in range(2):
                            nc.scalar.activation(
                                pband[:, hl, rel, v0:], ps2[:, hl, v0:],
                                EXP, scale=SCALE)
                            nc.gpsimd.affine_select(
                                out=pband[:, hl, rel, w0:w1],
                                in_=pband[:, hl, rel, w0:w1],
                                compare_op=mybir.AluOpType.is_ge, fill=0.0,
                                base=w0 - P * rel, channel_multiplier=-1,
                                pattern=[[0, 1], [1, w1 - w0]],
                            )
                    else:
                        nc.scalar.activation(
                            pband[:, :, rel, v0:], ps2[:, :, v0:], EXP,
                            scale=SCALE)
                        nc.gpsimd.affine_select(
                            out=pband[:, :, rel, w0:w1],
                            in_=pband[:, :, rel, w0:w1],
                            compare_op=mybir.AluOpType.is_ge, fill=0.0,
                            base=w0 - P * rel, channel_multiplier=-1,
                            pattern=[[0, 2], [1, w1 - w0]],
                        )
                    fill(1)

                # 2) leading full scores + filler bridge the band-exp chain
                lead = min(3 if OPTS.get("lead3") else 2, nfull)
                pps = {kt: scores_full(kt) for kt in range(lead)}
                fill(4)

                # 3) band AVs (rel0 covers all columns => carries start)
                for rel in range(4):
                    kt = 4 * qb + rel
                    av0 = min(P * rel, QW - 2 * P)
                    for hl in range(2):
                        h = 2 * g + hl
                        nc.tensor.matmul(
                            po[hl][:, av0:],
                            lhsT=vaug[:, kt, h * 65:(h + 1) * 65],
                            rhs=pband[:, hl, rel, av0:],
                            start=(rel == 0), stop=(rel == 3 and nfull == 0),
                        )
                    fill(1)

                # 4) remaining full tiles, scores->exp->AV with 1-filler gaps
                for kt in range(nfull):
                    if kt not in pps:
                        pps[kt] = scores_full(kt)
                    if kt + lead < nfull:
                        pps[kt + lead] = scores_full(kt + lead)
                    fill(1)
                    pp = pps.pop(kt)
                    for hl in range(2):
                        h = 2 * g + hl
                        nc.tensor.matmul(
                            po[hl],
                            lhsT=vaug[:, kt, h * 65:(h + 1) * 65],
                            rhs=pp[:, hl, :],
                            start=False, stop=(kt == nfull - 1),
                        )

                # copy po -> SBUF immediately so the PSUM buf frees after one
                # fast DVE op; the recip/bcast/mul chain then runs from SBUF
                # without stalling the next qb's AV (psO buf wait)
                ocp = osbp.tile([65, 2, QW], F32, tag="ocp",
                                name=f"ocp_{g}_{qb}")
                for hl in range(2):
                    nc.vector.tensor_copy(out=ocp[:, hl, :], in_=po[hl])
                for hl in range(2):
                    zrow = sp1.tile([1, QW], F32, tag="zrow",
                                    name=f"zr_{g}_{qb}_{hl}")
                    nc.vector.tensor_copy(out=zrow, in_=ocp[64:65, hl, :])
                    recip = sp1.tile([1, QW], F32, tag="recip",
                                     name=f"rc_{g}_{qb}_{hl}")
                    nc.vector.reciprocal_approx_fast(recip, zrow)
                    bc = sp.tile([64, QW], F32, tag="bc",
                                 name=f"bc_{g}_{qb}_{hl}")
                    nc.gpsimd.partition_broadcast(bc, recip)
                    nc.vector.tensor_mul(
                        out=oall[hl * 64:(hl + 1) * 64, g, qb * QW:(qb + 1) * QW],
                        in0=ocp[0:64, hl, :],
                        in1=bc,
                    )

                if g + 1 > NG - 1:
                    # g3: this qb's s-window of the output projection becomes
                    # filler for the remaining qbs
                    push_outproj_sw(qb)

            # hold remaining filler across the group boundary: the next
            # group's early qbs (esp. their band chains) are filler-starved;
            # drain_until guards enforce the proj deps each qb needs
            if g == NG - 1 or OPTS.get("no_holdback"):
                fill_all()

def _get_nc():
    global _NC
    if _NC is None:
        _NC = _build()
    return _NC


def _in_maps(x, w_qkv, w_proj):
    from ml_dtypes import bfloat16

    x = np.asarray(x, dtype=np.float32)
    w_qkv = np.asarray(w_qkv, dtype=np.float32)
    w_proj = np.asarray(w_proj, dtype=np.float32)

    maps = []
    for c in range(8):
        b, hh = c // 2, c % 2
        lo, hi = hh * 512, (hh + 1) * 512
        maps.append({
            "xT": np.ascontiguousarray(x[b].T).astype(bfloat16),
            "wqT": np.ascontiguousarray(w_qkv[lo:hi].T).astype(bfloat16),
            "wkT": np.ascontiguousarray(w_qkv[D + lo:D + hi].T).astype(bfloat16),
            "wvT": np.ascontiguousarray(w_qkv[2 * D + lo:2 * D + hi].T).astype(bfloat16),
            "wpT": np.ascontiguousarray(w_proj[:, lo:hi].T).astype(bfloat16),
        })
    return maps


def kernel(x, w_qkv, w_proj):
    from concourse.bass_utils import run_bass_kernel_spmd

    in_maps = _in_maps(x, w_qkv, w_proj)
    res = run_bass_kernel_spmd(_get_nc(), in_maps, core_ids=list(range(8)))
    out = np.empty((B, S, D), dtype=np.float32)
    for b in range(B):
        out[b] = (res.results[2 * b]["yT"].astype(np.float32)
                  + res.results[2 * b + 1]["yT"].astype(np.float32)).T
    return out

